# revision 18
# baseline (speedup 1.0000x reference)
"""BPCA pooling layer on 8 Trainium2 NeuronCores (Bass/Tile), fp16 edition.

Math: per sample, the reference's `data = patches.reshape(-1, 4)` groups 4
consecutive channels (C=256 divisible by 4), so `data` is the sample's
contiguous buffer viewed as [N, 4] with N = H*W*C/4:

  1. per-column mean/std over N rows, dn = (data-mean)/std
  2. gram = dn^T dn (4x4), comp = top eigenvector (jnp.linalg.eigh)
  3. out = (dn @ comp) reshaped to [H/2, W/2, C] with channel permutation
     c' = (2*di+dj)*64 + (c//4)

Both passes are HBM-bandwidth bound, so all device traffic is fp16: the
host quantizes x once to fp16 and BOTH the statistics and the projection
are computed from the quantized values (measured end-to-end deviation vs
the f32 reference: <=4.4e-4, versus the 2e-2 gate; the top-two eigengap of
the 4x4 gram is ~1.4e-3 minimum across samples, so the fp16-induced gram
perturbation rotates the principal direction by well under 1e-3).

Device plan (2 samples per core, pure data parallel):
  pass 1: only the 64 diagonal 4x4 blocks of the 256x256 channel
          second-moment matrix are needed (channel groups never straddle
          the half boundary), so the PE computes two half-grams
          M11=X1^T X1, M22=X2^T X2 ([128,130] each, with an interleaved
          ones column giving channel sums) instead of the full [256,258]
          product -- half the matmul columns of the v1 kernel, at fp16
          1 cyc/col. DMA is the bottleneck at ~13 MB/core.
  host:   fold M11/M22 into the 4x4 gram, mean/std/gram in f64, eigh on
          CPU jax (same implementation as the reference), w_k = comp_k/
          std_k, bias = -sum_k mean_k w_k.
  pass 2: out = sum_k w_k x_k + bias moves off the DVE onto the idle PE:
          partitions carry (k, pg) with pg 32 pixel-groups, lhsT is the
          block-structured [128, 32] weight matrix L[k*32+pg, m] =
          w_k*[pg==m], so one 512-col matmul computes 32 output rows;
          4 quarter-matmuls fill a [128, Fc] PSUM tile, one ACT pass adds
          the bias and casts to fp16, DVE queue issues the store. All
          engines hide under the fp16 DMA stream (~16 MB/core).
"""

import numpy as np

# ---------------------------------------------------------------------------
# Problem constants (hardcoded per spec)
# ---------------------------------------------------------------------------
B, H, W, C = 16, 112, 112, 256
N_CORES = 8
SPC = B // N_CORES          # samples per core = 2
PIX = H * W                 # 12544 pixels per sample
NBLK = PIX // 128           # 98 pixel-blocks of 128
BSTRIDE = 260               # per-block SBUF cols: 128 ch | 1 | pad | 128 ch | 1 | pad
NROWS = PIX * C // 4        # 802816 rows of the [N, 4] data matrix
HO, WO = H // 2, W // 2     # 56 x 56 output
OPP = NROWS // 128          # 6272 output cols per partition per sample
IPP = 4 * OPP               # 25088 input cols per partition per sample
FCS = [64, 192, 384] + [512] * 11   # pass-2 round sizes (sum = OPP)
assert sum(FCS) == OPP

_programs = None
LAST_PROFILE = {}
TRACE = False
TRACE_DIRS = {}


# ---------------------------------------------------------------------------
# TileContext with a walrus-compatible tail drain
# ---------------------------------------------------------------------------
def _make_tile_context(nc):
    from concourse.tile import TileContext
    return TileContext(nc)


def _split_sync_waits(nc):
    """walrus (CoreV2/V3 codegen) rejects instructions carrying more than 2
    sync commands (waits + updates combined); Tile freely emits e.g. 2 waits
    + 1 update.  Hoist excess waits onto same-engine NOPs inserted directly
    before the offending instruction -- same engine means the same program-
    order point, so semantics are unchanged."""
    import concourse.mybir as mybir

    def mint_nop(engine):
        inner = nc.engines[engine].nop().ins
        for blk in nc.m.functions[0].blocks:
            il = blk.instructions
            for k in range(len(il) - 1, -1, -1):
                if il[k] is inner:
                    il.pop(k)
                    return inner
        raise RuntimeError("minted nop not found in any block")

    for fn in nc.m.functions:
        for blk in fn.blocks:
            il = blk.instructions
            i = 0
            while i < len(il):
                inst = il[i]
                si = inst.sync_info
                waits = list(si.on_wait) if si and si.on_wait else []
                upds = list(si.on_update) if si and si.on_update else []
                # observed walrus limits: at most 1 wait per instruction
                # (1 wait + 1 update compiles; 2 waits anywhere does not)
                if len(waits) > 1:
                    extra, keep = waits[:-1], waits[-1:]
                    for wchunk in extra:
                        nop = mint_nop(inst.engine)
                        nop.sync_info = mybir.SyncInfo(
                            on_wait=[wchunk], on_update=[])
                        il.insert(i, nop)
                        i += 1
                    inst.sync_info = mybir.SyncInfo(
                        on_wait=keep, on_update=upds)
                i += 1


def _build_pass1():
    import concourse.bass as bass
    import concourse.mybir as mybir

    f16 = mybir.dt.float16
    f32 = mybir.dt.float32

    nc = bass.Bass("TRN2", target_bir_lowering=False, debug=False,
                   num_devices=N_CORES)
    # Block layout (host-built): [ch0..127 | ones | pad | ch128..255 | ones
    # | pad] = 260 fp16 cols, so both half-matmuls read 4-byte-aligned
    # contiguous slices and the ones column rides the same DMA.
    x = nc.dram_tensor("x", [SPC, 128, NBLK * BSTRIDE], f16,
                       kind="ExternalInput").ap()
    stats = nc.dram_tensor("stats", [SPC, 128, 260], f32,
                           kind="ExternalOutput").ap()

    with _make_tile_context(nc) as tc:
        with (
            tc.tile_pool(name="inp", bufs=6) as inp,
            tc.tile_pool(name="psum", bufs=2, space="PSUM") as psum,
            tc.tile_pool(name="sout", bufs=2) as soutp,
        ):
            qi = 0
            for s in range(SPC):
                ps1 = psum.tile([128, 130], f32, tag="ps1")
                ps2 = psum.tile([128, 130], f32, tag="ps2")
                # graduated tile sizes: tiny first tile so the PE starts
                # early instead of waiting on a full-size load; few, large
                # DMAs (each dma_start costs ~0.6us of engine issue time)
                b0 = 0
                for nb in [2, 12, 21, 21, 21, 21]:
                    t = inp.tile([128, nb * BSTRIDE], f16, tag="in")
                    t3 = t[:].rearrange("p (j b) -> p j b", b=BSTRIDE)
                    # alternate the two hardware DMA queues (SP- and
                    # ACT-issued) so loads stream from both in parallel
                    eng = nc.sync if qi % 2 == 0 else nc.scalar
                    qi += 1
                    eng.dma_start(
                        out=t[:],
                        in_=x[s, :, b0 * BSTRIDE:(b0 + nb) * BSTRIDE])
                    for j in range(nb):
                        first = b0 + j == 0
                        last = b0 + j == NBLK - 1
                        nc.tensor.matmul(ps1[:, 0:130],
                                         t3[:, j:j + 1, 0:128],
                                         t3[:, j:j + 1, 0:130],
                                         start=first, stop=last,
                                         skip_group_check=True)
                        nc.tensor.matmul(ps2[:, 0:130],
                                         t3[:, j:j + 1, 130:258],
                                         t3[:, j:j + 1, 130:260],
                                         start=first, stop=last,
                                         skip_group_check=True)
                    b0 += nb
                so = soutp.tile([128, 260], f32)
                nc.vector.tensor_copy(out=so[:, 0:130], in_=ps1[:, 0:130])
                nc.vector.tensor_copy(out=so[:, 130:260], in_=ps2[:, 0:130])
                # ACT-issued DMA: keeps the SP queue free to prefetch the
                # next sample's tiles
                nc.scalar.dma_start(out=stats[s], in_=so[:])
    _split_sync_waits(nc)
    return nc


def _build_pass2():
    import concourse.bass as bass
    import concourse.mybir as mybir

    f16 = mybir.dt.float16
    f32 = mybir.dt.float32
    Ident = mybir.ActivationFunctionType.Identity

    nc = bass.Bass("TRN2", target_bir_lowering=False, debug=False,
                   num_devices=N_CORES)
    # Partition p = k*32 + pg; per sample each partition carries OPP output
    # elements (free axis), host-packed per round r as [rhs_q0|..|rhs_q3]
    # with rhs_q[k*32+pg, c] = xk[(q*32+pg)*OPP + o_r + c].
    x = nc.dram_tensor("x", [128, SPC * IPP], f16,
                       kind="ExternalInput").ap()
    wm = nc.dram_tensor("wm", [128, SPC * 32], f16,
                        kind="ExternalInput").ap()
    bv = nc.dram_tensor("bv", [128, SPC], f32, kind="ExternalInput").ap()
    out = nc.dram_tensor("out", [128, SPC * OPP], f16,
                         kind="ExternalOutput").ap()

    alu = mybir.AluOpType
    # rounds grouped so each DMA (load AND store) covers a whole group:
    # dma_start costs ~0.6us of engine issue time, so few big transfers win
    GROUPS = [[64, 192, 384], [512] * 4, [512] * 4, [512] * 3]
    with _make_tile_context(nc) as tc:
        with (
            tc.tile_pool(name="w", bufs=1) as wpool,
            tc.tile_pool(name="inp", bufs=3) as inp,
            tc.tile_pool(name="psum", bufs=3, space="PSUM") as psum,
            tc.tile_pool(name="ot", bufs=3) as outp,
        ):
            wt = wpool.tile([128, SPC * 32], f16, tag="wm")
            nc.sync.dma_start(out=wt[:], in_=wm[:])
            bt = wpool.tile([128, SPC], f32, tag="bv")
            nc.sync.dma_start(out=bt[:], in_=bv[:])
            gi = 0
            for s in range(SPC):
                off = s * IPP
                ooff = s * OPP
                for grp in GROUPS:
                    gin = 4 * sum(grp)
                    gout = sum(grp)
                    it = inp.tile([128, gin], f16, tag="it")
                    # alternate SP-/ACT-issued DMAs across both hardware
                    # queues (group parity): loads and stores of the same
                    # group go to opposite queues, so both stream ~8MB
                    gi += 1
                    ldq = nc.sync if gi % 2 == 0 else nc.scalar
                    ldq.dma_start(out=it[:], in_=x[:, off:off + gin])
                    ot = outp.tile([128, gout], f16, tag="ot")
                    goff = 0
                    for fc in grp:
                        ps = psum.tile([128, fc], f32, tag="ps")
                        for q in range(4):
                            nc.tensor.matmul(
                                ps[q * 32:(q + 1) * 32, 0:fc],
                                wt[:, s * 32:(s + 1) * 32],
                                it[:, 4 * goff + q * fc:
                                   4 * goff + (q + 1) * fc],
                                start=True, stop=True,
                                skip_group_check=True,
                                tile_position=(0, q * 32))
                        # bias + fp16 cast on the otherwise idle DVE
                        nc.vector.tensor_scalar(
                            ot[:, goff:goff + fc], ps[:, 0:fc], 1.0,
                            bt[:, s:s + 1], op0=alu.mult, op1=alu.add)
                        goff += fc
                    # gpsimd (software DGE) stores: both hardware queues
                    # stay pure-load, so a store waiting on compute never
                    # head-of-line-blocks a prefetch
                    nc.gpsimd.dma_start(out=out[:, ooff:ooff + gout],
                                        in_=ot[:])
                    off += gin
                    ooff += gout
    _split_sync_waits(nc)
    return nc


def _get_programs():
    global _programs
    if _programs is None:
        _programs = (_build_pass1(), _build_pass2())
    return _programs


def _host_middle(stats):
    """stats: [B, 128, 260] f32 -> w [B, 4] f64, bias [B] f64.

    stats[:, m, 0:128]   = M11[ch m, ch j],  stats[:, m, 128]   = sum(ch m)
    stats[:, m, 130:258] = M22[ch 128+m, ch 128+j], stats[:, m, 258] = sum.
    Follows the reference downstream exactly: gram from (S - N mu mu^T) /
    (sigma sigma^T), comp = eigh(gram f32) top eigenvector on CPU jax.
    """
    stats = stats.astype(np.float64)
    M11 = stats[:, :, 0:128]
    M22 = stats[:, :, 130:258]
    cs = stats[:, :, 128] + stats[:, :, 258]        # [B, 128] folded halves

    # fold channels c = 4a+k into columns k (diagonal 4x4 blocks only)
    S = (np.einsum("bgkgl->bkl", M11.reshape(B, 32, 4, 32, 4))
         + np.einsum("bgkgl->bkl", M22.reshape(B, 32, 4, 32, 4)))
    colsum = cs.reshape(B, 32, 4).sum(axis=1)       # [B, 4]

    mu = colsum / NROWS
    e2 = np.einsum("bkk->bk", S) / NROWS
    var = np.maximum(e2 - mu * mu, 0.0)
    sigma = np.sqrt(var)
    denom = sigma[:, :, None] * sigma[:, None, :]
    gram = (S - NROWS * mu[:, :, None] * mu[:, None, :])
    with np.errstate(divide="ignore", invalid="ignore"):
        gram = np.where(denom > 0, gram / np.where(denom > 0, denom, 1.0), 0.0)

    # eigh with the same implementation/backend the reference uses (CPU jax)
    import jax
    import jax.numpy as jnp
    with jax.default_device(jax.devices("cpu")[0]):
        V = np.asarray(jnp.linalg.eigh(jnp.asarray(gram, jnp.float32))[1])
    comp = V[:, :, -1].astype(np.float64)                # top eigenvector

    with np.errstate(divide="ignore", invalid="ignore"):
        w = np.where(sigma > 0, comp / np.where(sigma > 0, sigma, 1.0), 0.0)
    bias = -(mu * w).sum(axis=1)
    return w, bias


def _prep_pass1(xq):
    """xq: [B, PIX, C] fp16 -> [B, 128, NBLK*BSTRIDE] fp16 block layout."""
    xp = np.zeros((B, 128, NBLK, BSTRIDE), np.float16)
    xb = xq.reshape(B, NBLK, 128, C).transpose(0, 2, 1, 3)
    xp[..., 0:128] = xb[..., 0:128]
    xp[..., 128] = 1.0
    xp[..., 130:258] = xb[..., 128:256]
    xp[..., 258] = 1.0
    return xp.reshape(B, 128, NBLK * BSTRIDE)


def _prep_pass2(xq):
    """xq: [B, PIX, C] fp16 -> [B, 128, IPP] fp16 quarter-packed k-planes."""
    # xk2d[b, k, p, j] = k-plane value for output element n = p*OPP + j
    xk2d = xq.reshape(B, NROWS, 4).transpose(0, 2, 1).reshape(B, 4, 128, OPP)
    x2 = np.empty((B, 4, 32, IPP), np.float16)
    o = 0
    for fc in FCS:
        seg = xk2d[:, :, :, o:o + fc].reshape(B, 4, 4, 32, fc)
        # seg[b, k, q, pg, c] -> row k*32+pg, cols [4*o + q*fc + c]
        x2[:, :, :, 4 * o:4 * (o + fc)] = (
            seg.transpose(0, 1, 3, 2, 4).reshape(B, 4, 32, 4 * fc))
        o += fc
    return x2.reshape(B, 128, IPP)


def _unscramble_out(o):
    """o: [SPC, 128, OPP] f32 -> [SPC, HO, WO, C]."""
    # element (p, j) is output n = p*OPP + j with n = pix*64 + g
    o = o.reshape(SPC, PIX, 64).reshape(SPC, HO, 2, WO, 2, 64)
    return o.transpose(0, 1, 3, 2, 4, 5).reshape(SPC, HO, WO, C)


def kernel(x):
    from concourse.bass_utils import run_bass_kernel_spmd

    x = np.asarray(x)
    assert x.shape == (B, H, W, C), x.shape
    xq = np.ascontiguousarray(x, dtype=np.float16).reshape(B, PIX, C)
    nc1, nc2 = _get_programs()
    core_ids = list(range(N_CORES))

    xp = _prep_pass1(xq)
    in1 = [{"x": xp[c * SPC:(c + 1) * SPC]} for c in range(N_CORES)]
    kw1 = dict(trace=True, tmpdir=TRACE_DIRS.get("pass1")) if TRACE else {}
    r1 = run_bass_kernel_spmd(nc1, in1, core_ids, **kw1)
    if TRACE:
        LAST_PROFILE["pass1_ns"] = r1.exec_time_ns
    stats = np.concatenate([r1.results[c]["stats"] for c in range(N_CORES)])

    w, bias = _host_middle(stats)
    x2 = _prep_pass2(xq)
    in2 = []
    for c in range(N_CORES):
        wmc = np.zeros((128, SPC * 32), np.float16)
        bvc = np.zeros((128, SPC), np.float32)
        for s in range(SPC):
            b = c * SPC + s
            for k in range(4):
                wmc[k * 32:(k + 1) * 32, s * 32:(s + 1) * 32] = (
                    np.eye(32, dtype=np.float32) * np.float32(w[b, k])
                ).astype(np.float16)
            bvc[:, s] = np.float32(bias[b])
        pair = x2[c * SPC:(c + 1) * SPC]
        in2.append({"x": np.ascontiguousarray(
            pair.transpose(1, 0, 2).reshape(128, SPC * IPP)),
            "wm": wmc, "bv": bvc})
    kw2 = dict(trace=True, tmpdir=TRACE_DIRS.get("pass2")) if TRACE else {}
    r2 = run_bass_kernel_spmd(nc2, in2, core_ids, **kw2)
    if TRACE:
        LAST_PROFILE["pass2_ns"] = r2.exec_time_ns

    outs = []
    for c in range(N_CORES):
        o = r2.results[c]["out"].astype(np.float32)     # [128, SPC*OPP]
        o = o.reshape(128, SPC, OPP).transpose(1, 0, 2)  # [SPC, 128, OPP]
        outs.append(_unscramble_out(o))
    return np.ascontiguousarray(np.concatenate(outs))
